# revision 43
# baseline (speedup 1.0000x reference)
"""AttnBlock3D Trainium2 Bass kernel (8 NeuronCores, SPMD).

Layout / algorithm (per core r, heads n = 2r, 2r+1):
  x viewed as [128=(t,c), 4096=hw].  BN stats computed on-device (sum
  reduce on DVE, x^2 on GPSIMD, per-channel combine + [16]->[128]
  partition broadcast via tiny selector matmuls); gamma/beta and conv
  biases folded on the host into block-diagonal projection weights.
  Attention is tiled as 8 i-windows of 512 x 32 j-tiles of 128.  Per
  j-tile both heads' QK matmuls write one [128, 1024] "pair tile" of PSUM
  (2 banks, h0 cols 0:512 / h1 cols 512:1024, concurrent via PE row
  groups 0/1).  The exp of each pair tile - the kernel's dominant cost,
  33.6M score elements per core that must leave PSUM through the 1
  elem/cycle/lane engine ports - is split across BOTH the ACT engine
  (exact Exp) and the DVE (Schraudolph approx: int16(round(s*A + B))
  bitcast to bf16 ~= exp(s*SCALE) to ~3%, which softmax normalization
  cancels to ~5e-5 end-to-end): every 4th tile goes wholly to ACT, the
  rest are split h0-bank->ACT / h1-bank->DVE running concurrently
  (different PSUM banks), giving ACT ~2/3 of the elements to match the
  engines' effective rates (the DVE pays a pipe-drain ~= its op duration
  between ops).  QK psum is triple buffered (6 banks) and AV runs two
  j-tiles behind exp so the PE queue head never blocks on a semaphore.
  AV: both heads accumulate into ONE [128, 512] psum bank via column
  groups 0/32 (h0 -> partitions 0-8, h1 -> 32-40; partitions 0/32 =
  sumexp via the ones column of the t9 lhsT), double buffered across
  windows.  Unnormalized sums + sumexp are AllGather'd in 4 chunks (one
  per window pair) so the collectives overlap attention; normalization +
  wp projection + residual run per chunk as gathers land (reciprocal on
  DVE, psum evacuation + bias on ACT, multiply/residual on GPSIMD).
"""
import sys

import numpy as np

sys.path.insert(0, "/opt/trn_rl_repo")

T, C, HW, NCORES = 8, 16, 4096, 8
N_ELEM = T * HW  # per-channel element count for BN stats
EPS = 1e-5
SCALE = float(T) ** -0.5
NWIN = 8            # i-windows of 512
NJT = 32            # j-tiles of 128
# exp work split: on jt % 4 == 0 the ACT does the whole [128,1024] pair
# tile; otherwise ACT takes h0's bank and the DVE takes h1's bank in
# parallel (different PSUM banks -> concurrent access is legal).  This
# gives ACT ~66% of the elements (its drain-free rate is ~2x the DVE's
# drain-inclusive rate) without ever queuing two ops on one engine.
ACT_FULL_EVERY = 4
# Schraudolph: round(s*A + B) as int16, bitcast bf16 ~= exp(s*SCALE)
EXP_A = SCALE * 128.0 / float(np.log(2.0))
EXP_B = 16250.5

_CACHE = {}


def _act_full(jt):
    """True -> ACT handles the whole pair tile; else ACT h0 / DVE h1."""
    return jt % ACT_FULL_EVERY == 0


def _build_program():
    import concourse.bass as bass
    import concourse.bacc as bacc
    import concourse.tile as tile
    from concourse import mybir

    f32 = mybir.dt.float32
    bf16 = mybir.dt.bfloat16
    i16 = mybir.dt.int16
    AX = mybir.AxisListType
    OP = mybir.AluOpType
    ACT = mybir.ActivationFunctionType

    nc = bacc.Bacc("TRN2", target_bir_lowering=False, debug=False,
                   num_devices=NCORES)
    x = nc.dram_tensor("x", [128, HW], f32, kind="ExternalInput").ap()
    wq_bd = nc.dram_tensor("wq_bd", [128, 64], bf16, kind="ExternalInput").ap()
    wk_bd = nc.dram_tensor("wk_bd", [128, 64], bf16, kind="ExternalInput").ap()
    wv_rhs = nc.dram_tensor("wv_rhs", [128, 18], bf16,
                            kind="ExternalInput").ap()
    bq_col = nc.dram_tensor("bq_col", [64, 1], f32, kind="ExternalInput").ap()
    bk_col = nc.dram_tensor("bk_col", [64, 1], f32, kind="ExternalInput").ap()
    wp_bd = nc.dram_tensor("wp_bd", [128, 128], bf16,
                           kind="ExternalInput").ap()
    bp_col = nc.dram_tensor("bp_col", [128, 1], f32, kind="ExternalInput").ap()
    sel = nc.dram_tensor("sel", [128, 16], f32, kind="ExternalInput").ap()
    selT = nc.dram_tensor("selT", [16, 128], f32, kind="ExternalInput").ap()
    out = nc.dram_tensor("out", [128, HW], f32, kind="ExternalOutput").ap()

    # per-window-pair collective buffers (chunked AllGather overlaps compute)
    cc_in = [nc.dram_tensor(f"cc_in{p}", [18, 1024], f32).ap()
             for p in range(4)]
    cc_out = [nc.dram_tensor(f"cc_out{p}", [NCORES * 18, 1024], f32,
                             addr_space="Shared").ap()
              for p in range(4)]

    with tile.TileContext(nc) as tc:
        with (
            tc.tile_pool(name="persist", bufs=1) as P1,
            tc.tile_pool(name="work", bufs=5) as PW,
            tc.tile_pool(name="scratch", bufs=1) as PS,
            tc.tile_pool(name="psq", bufs=3, space="PSUM") as PSQ,
            tc.tile_pool(name="psa", bufs=2, space="PSUM") as PSA,
            tc.tile_pool(name="dram", bufs=1, space="DRAM") as PD,
        ):
            # ---------------- loads ----------------
            x_sb = P1.tile([128, HW], f32)
            for qi, eng in enumerate((nc.sync, nc.gpsimd, nc.scalar,
                                      nc.sync)):
                cs = slice(qi * 1024, (qi + 1) * 1024)
                eng.dma_start(out=x_sb[:, cs], in_=x[:, cs])
            wqbd_sb = P1.tile([128, 64], bf16)
            nc.scalar.dma_start(out=wqbd_sb, in_=wq_bd)
            wkbd_sb = P1.tile([128, 64], bf16)
            nc.scalar.dma_start(out=wkbd_sb, in_=wk_bd)
            wvrhs_sb = P1.tile([128, 18], bf16)
            nc.scalar.dma_start(out=wvrhs_sb, in_=wv_rhs)
            bqcol_sb = P1.tile([64, 1], f32)
            nc.gpsimd.dma_start(out=bqcol_sb, in_=bq_col)
            bkcol_sb = P1.tile([64, 1], f32)
            nc.gpsimd.dma_start(out=bkcol_sb, in_=bk_col)
            wpbd_sb = P1.tile([128, 128], bf16)
            nc.scalar.dma_start(out=wpbd_sb, in_=wp_bd)
            bpcol_sb = P1.tile([128, 1], f32)
            nc.sync.dma_start(out=bpcol_sb, in_=bp_col)
            sel_sb = P1.tile([128, 16], f32)
            nc.sync.dma_start(out=sel_sb, in_=sel)
            selT_sb = P1.tile([16, 128], f32)
            nc.sync.dma_start(out=selT_sb, in_=selT)

            # ---------------- BN stats ----------------
            # xsq on gpsimd in parallel with the DVE's sum-reduce
            s1 = P1.tile([128, 2], f32)
            xsq = PS.tile([128, HW], f32, tag="xsq")
            nc.gpsimd.tensor_mul(xsq, x_sb, x_sb)
            nc.vector.reduce_sum(out=s1[:, 0:1], in_=x_sb, axis=AX.X)
            nc.vector.reduce_sum(out=s1[:, 1:2], in_=xsq, axis=AX.X)

            # per-channel sums on partitions 0-15: [16, (sum, sumsq)]
            ps_st = PSA.tile([16, 2], f32, tag="av")
            nc.tensor.matmul(ps_st, lhsT=sel_sb, rhs=s1, start=True,
                             stop=True)
            stc = P1.tile([16, 2], f32)
            nc.vector.tensor_scalar_mul(stc, ps_st, 1.0 / N_ELEM)
            var = P1.tile([16, 1], f32)
            nc.vector.tensor_mul(var, stc[:, 0:1], stc[:, 0:1])
            nc.vector.tensor_sub(var, stc[:, 1:2], var)
            eps_t = P1.tile([16, 1], f32)
            nc.vector.memset(eps_t, EPS)
            zero16 = P1.tile([16, 1], f32)
            nc.vector.memset(zero16, 0.0)
            m2 = P1.tile([16, 2], f32)
            nc.vector.tensor_copy(m2[:, 0:1], stc[:, 0:1])
            nc.scalar.activation(m2[:, 1:2], var, ACT.Ln, bias=eps_t)
            nc.scalar.activation(m2[:, 1:2], m2[:, 1:2], ACT.Exp, scale=-0.5,
                                 bias=zero16)
            # broadcast [16,2] -> [128,2] with the transposed selector
            ps_mi = PSA.tile([128, 2], f32, tag="av")
            nc.tensor.matmul(ps_mi, lhsT=selT_sb, rhs=m2, start=True,
                             stop=True)
            mi = P1.tile([128, 2], f32)
            nc.vector.tensor_copy(mi, ps_mi)
            xhat = P1.tile([128, HW], bf16)
            nc.vector.tensor_scalar(out=xhat, in0=x_sb, scalar1=mi[:, 0:1],
                                    scalar2=mi[:, 1:2], op0=OP.subtract,
                                    op1=OP.mult)

            # ---------------- q/k projections (bf16) ----------------
            # k bias-adds on ACT, q bias-adds on DVE (parallel prologue)
            q_sb = P1.tile([64, HW], bf16)
            k_sb = P1.tile([64, HW], bf16)
            for ch in range(HW // 512):
                cs = slice(ch * 512, (ch + 1) * 512)
                psk = PSQ.tile([64, 512], f32, tag="qk", name="pk")
                nc.tensor.matmul(psk, lhsT=wkbd_sb, rhs=xhat[:, cs],
                                 start=True, stop=True)
                nc.scalar.activation(k_sb[:, cs], psk, ACT.Identity,
                                     bias=bkcol_sb)
                psq = PSQ.tile([64, 512], f32, tag="qk", name="pq")
                nc.tensor.matmul(psq, lhsT=wqbd_sb, rhs=xhat[:, cs],
                                 start=True, stop=True)
                nc.vector.tensor_scalar_add(out=q_sb[:, cs], in0=psq,
                                            scalar1=bqcol_sb)

            # ---------------- v -> t9 (bf16, ones in cols 0 and 9) -------
            # t9[:, jc, 0:9] = [ones | vT h0], t9[:, jc, 9:18] = [ones | vT h1]
            t9 = P1.tile([128, 32, 18], bf16)
            def build_t9(jc):
                psv = PSA.tile([128, 18], f32, tag="av", name="psv")
                nc.tensor.matmul(psv, lhsT=xhat[:, jc * 128:(jc + 1) * 128],
                                 rhs=wvrhs_sb, start=True, stop=True)
                if jc % 2 == 0:
                    nc.scalar.copy(out=t9[:, jc, 1:9], in_=psv[:, 1:9])
                    nc.scalar.copy(out=t9[:, jc, 10:18], in_=psv[:, 10:18])
                else:
                    nc.vector.tensor_copy(t9[:, jc, 1:9], psv[:, 1:9])
                    nc.vector.tensor_copy(t9[:, jc, 10:18], psv[:, 10:18])

            nc.vector.memset(t9[:, :, 0:1], 1.0)
            nc.vector.memset(t9[:, :, 9:10], 1.0)
            for jc in range(32):
                build_t9(jc)

            # ---------------- attention ----------------
            zero128 = P1.tile([128, 1], f32)
            nc.vector.memset(zero128, 0.0)

            def tail_chunk(p):
                """Normalize + project + residual for gathered chunk p
                (i-cols [p*1024, (p+1)*1024))."""
                ccp = cc_out[p]
                # sumexp rows 9n -> rsum[n*8+g, 0:128] (g = 128-col block)
                rsum = PW.tile([128, 128], f32, tag="rsum")
                src = bass.AP(tensor=ccp.tensor, offset=ccp.offset,
                              ap=[[9 * 1024, 16], [128, 8], [1, 128]])
                nc.sync.dma_start(out=rsum[:], in_=src)
                rinv = PW.tile([128, 128], f32, tag="rinv")
                nc.vector.reciprocal(rinv, rsum)
                rd = PD.tile([16, 1024], f32, tag=f"rd{p}")
                rd_t = rd[:].tensor
                dst = bass.AP(tensor=rd_t, offset=rd[:].offset,
                              ap=[[1024, 16], [128, 8], [1, 128]])
                nc.sync.dma_start(out=dst, in_=rinv[:])
                for ch in range(2):
                    c0 = ch * 512
                    rbc = PW.tile([128, 512], f32, tag="rbc")
                    src2 = bass.AP(tensor=rd_t, offset=rd[:].offset + c0,
                                   ap=[[1024, 16], [0, T], [1, 512]])
                    nc.sync.dma_start(out=rbc[:], in_=src2)
                    acf = PW.tile([128, 512], f32, tag="acf")
                    src3 = bass.AP(tensor=ccp.tensor,
                                   offset=ccp.offset + 1024 + c0,
                                   ap=[[9 * 1024, 16], [1024, T], [1, 512]])
                    nc.scalar.dma_start(out=acf[:], in_=src3)
                    att_n = PW.tile([128, 512], bf16, tag="att_n")
                    nc.gpsimd.tensor_mul(att_n, acf, rbc)
                    psp = PSQ.tile([128, 512], f32, tag="qk", name="psp")
                    nc.tensor.matmul(psp, lhsT=wpbd_sb, rhs=att_n,
                                     start=True, stop=True)
                    och = PW.tile([128, 512], f32, tag="och")
                    nc.scalar.activation(och, psp, ACT.Identity,
                                         bias=bpcol_sb)
                    och2 = PW.tile([128, 512], f32, tag="och2")
                    oc = p * 1024 + c0
                    nc.gpsimd.tensor_add(och2, och, x_sb[:, oc:oc + 512])
                    nc.sync.dma_start(out=out[:, oc:oc + 512], in_=och2)

            for w in range(NWIN):
                i0 = w * 512
                av = PSA.tile([128, 512], f32, tag="av", name=f"av{w}")
                # AV runs TWO j-tiles behind QK/exp; with the bank-split
                # exp, both engines' halves are consumer-visible well
                # within 2 periods, so the PE queue head never blocks.
                exs = [None, None]  # ex tiles for jt-1, jt-2
                for jt in range(NJT + 2):
                    ex_cur = None
                    if jt < NJT:
                        qk = PSQ.tile([128, 1024], f32, tag="qk",
                                      name=f"qk{w}_{jt}")
                        for l in range(2):
                            nc.tensor.matmul(
                                qk[:, l * 512:(l + 1) * 512],
                                lhsT=k_sb[l * 32:l * 32 + 8,
                                          jt * 128:(jt + 1) * 128],
                                rhs=q_sb[l * 32:l * 32 + 8, i0:i0 + 512],
                                start=True, stop=True)
                        ex = PW.tile([128, 1024], bf16, tag="ex")
                        if _act_full(jt):
                            nc.scalar.activation(ex, qk, ACT.Exp,
                                                 scale=SCALE, bias=zero128)
                        else:
                            nc.scalar.activation(ex[:, 0:512],
                                                 qk[:, 0:512], ACT.Exp,
                                                 scale=SCALE, bias=zero128)
                            nc.vector.tensor_scalar(
                                out=ex[:, 512:1024].bitcast(i16),
                                in0=qk[:, 512:1024],
                                scalar1=EXP_A, scalar2=EXP_B,
                                op0=OP.mult, op1=OP.add)
                        ex_cur = ex
                    if jt >= 2:
                        for l in range(2):
                            nc.tensor.matmul(
                                av[32 * l:32 * l + 9, :],
                                lhsT=t9[:, jt - 2, 9 * l:9 * l + 9],
                                rhs=exs[1][:, l * 512:(l + 1) * 512],
                                start=(jt == 2), stop=(jt == NJT + 1),
                                tile_position=(0, 32 * l),
                                skip_group_check=True)
                    exs = [ex_cur, exs[0]]
                # ship unnormalized rows + sumexp (h0 -> rows 0:9,
                # h1 -> rows 9:18 of the window-pair chunk)
                s128 = PW.tile([128, 512], f32, tag="s128")
                nc.vector.tensor_copy(s128[0:41, :], av[0:41, :])
                p, half = w // 2, (w % 2) * 512
                nc.sync.dma_start(out=cc_in[p][0:9, half:half + 512],
                                  in_=s128[0:9, :])
                nc.sync.dma_start(out=cc_in[p][9:18, half:half + 512],
                                  in_=s128[32:41, :])
                if w % 2 == 1:
                    nc.gpsimd.collective_compute(
                        "AllGather", OP.bypass,
                        replica_groups=[list(range(NCORES))],
                        ins=[cc_in[p].opt()], outs=[cc_out[p].opt()])
                # emit tail work late enough that its AllGather is done
                # (no engine-FIFO stall), early enough to overlap compute
                if w == 5:
                    tail_chunk(0)
                elif w == 7:
                    tail_chunk(1)
            tail_chunk(2)
            tail_chunk(3)

    nc.compile()
    return nc


def host_inputs(r, x128, gamma, beta, wq, bq, wk, bk, wv, bv, wp, bp):
    """Per-core host-side input prep (folds gamma/beta/biases)."""
    import ml_dtypes
    bf = ml_dtypes.bfloat16
    wq_e = (wq * gamma[None, :]).astype(np.float32)
    wk_e = (wk * gamma[None, :]).astype(np.float32)
    wv_e = (wv * gamma[None, :]).astype(np.float32)
    bq_e = (bq + wq @ beta).astype(np.float32)
    bk_e = (bk + wk @ beta).astype(np.float32)
    bv_e = (bv + wv @ beta).astype(np.float32)
    bp_e = (bp + wp @ bv_e).astype(np.float32)

    wq_bd = np.zeros((128, 64), np.float32)
    wk_bd = np.zeros((128, 64), np.float32)
    wv_rhs = np.zeros((128, 18), np.float32)
    bq_col = np.zeros((64, 1), np.float32)
    bk_col = np.zeros((64, 1), np.float32)
    fi = np.arange(T)
    ci = np.arange(C)
    for l in range(2):
        n = 2 * r + l
        wq_bd[fi[:, None] * 16 + ci[None, :], (l * 32 + fi)[:, None]] = wq_e[n]
        wk_bd[fi[:, None] * 16 + ci[None, :], (l * 32 + fi)[:, None]] = wk_e[n]
        wv_rhs[fi[:, None] * 16 + ci[None, :],
               (l * 9 + 1 + fi)[:, None]] = wv_e[n]
        bq_col[l * 32 + fi, 0] = bq_e[n]
        bk_col[l * 32 + fi, 0] = bk_e[n]
    # p-conv lhsT rows are in (c,f) order to match the gathered layout
    wp_bd = np.zeros((128, 128), np.float32)
    bp_col = np.zeros((128, 1), np.float32)
    for f in range(T):
        wp_bd[np.ix_(ci * 8 + f, f * 16 + ci)] = wp.T
        bp_col[f * 16 + ci, 0] = bp_e
    selm = np.zeros((128, 16), np.float32)
    selm[np.arange(128), np.tile(ci, T)] = 1.0
    return dict(x=x128, wq_bd=wq_bd.astype(bf), wk_bd=wk_bd.astype(bf),
                wv_rhs=wv_rhs.astype(bf), bq_col=bq_col, bk_col=bk_col,
                wp_bd=wp_bd.astype(bf), bp_col=bp_col, sel=selm,
                selT=np.ascontiguousarray(selm.T))


def make_in_maps(inputs):
    x = np.ascontiguousarray(np.asarray(inputs["x"], np.float32))
    x128 = x.reshape(128, HW)
    args = {k: np.asarray(v, np.float32) for k, v in inputs.items()
            if k != "x"}
    return [host_inputs(r, x128, **args) for r in range(NCORES)]


def run(inputs, trace=False):
    """Returns (out (8,16,64,64) f32, BassKernelResults)."""
    from concourse.bass_utils import run_bass_kernel_spmd
    if "nc" not in _CACHE:
        _CACHE["nc"] = _build_program()
    nc = _CACHE["nc"]
    in_maps = make_in_maps(inputs)
    res = run_bass_kernel_spmd(nc, in_maps, list(range(NCORES)), trace=trace)
    out = np.asarray(res.results[0]["out"], np.float32).reshape(T, C, 64, 64)
    return out, res


def kernel(**inputs):
    out, _ = run(inputs, trace=False)
    return out
